# revision 22
# baseline (speedup 1.0000x reference)
"""Multi-head attention (B=4, N=2048, C=768, H=12, D=64) on 8 TRN2 NeuronCores.

Sharding: core c handles batch b=c//2 and half the heads (6 heads, g=c%2).
Per core: q/k/v projections for its head slice, S^T-layout attention (nk on
partitions, nq on free), softmax on the Scalar engine only (exp over
[128, 1024] tiles covering both heads of a pair), PV with V as the stationary
operand extended by a ones column (M=65) so the softmax denominator Z
accumulates for free in PSUM row 64, per-(pair,chunk,head) 1/Z via
reciprocal_approx_fast directly on the PSUM Z row, a DRAM-bounce partition
broadcast of 1/Z, fused (evict x 1/Z-scale) of the PV accumulator, and a
partial output projection.  Host sums the two per-batch partials and adds bo.

Relative to the previous version this removes ALL non-exp work from the
Scalar engine (exp is the engine roofline), removes the DVE Z-accumulation
tree and the [1,N] RECIPROCAL (79us profiled) entirely, and drops the
ones-vector Z matmuls from the PE.  qT/kT are bf16 (same PE rate as f32r,
half the eviction/SBUF cost); scores remain ~N(0,64) so bf16 rounding adds
<1% exp error, well inside the 2e-2 gate.

Layout notes: CHUNK=512 (4 chunks); st tiles are [128, 1024] f32 PSUM
(2 banks, h0 cols 0:512, h1 cols 512:1024) written by two tile_position
matmuls and consumed by ONE activation; pv tiles are per-head [65, 512] f32
(1 bank); v_sb rows are [128, 6*68] bf16 with head j at cols 68j..68j+63 and
a ones column at 68j+64 (68-stride keeps 4B alignment).
"""

import numpy as np

B, N, C = 4, 2048, 768
H, D = 12, 64
HPC = 6                 # heads per core
DV = HPC * D            # 384
P = 128
KC = C // P             # 6 contraction chunks for projections
NPAIR = 3               # head-pairs per core
NT = N // P             # 16 nk tiles
CHUNK = 512
NCH = N // CHUNK        # 4 chunks
VST = 68                # v_sb per-head stride (64 data + 1 ones + 3 pad)
SCALE = 1.0 / np.sqrt(D)

_CACHE = {}


def _build(reps=1):
    import warnings
    warnings.filterwarnings("ignore")
    import concourse.bass as bass
    import concourse.bacc as bacc
    import concourse.mybir as mybir
    from concourse import tile

    f32 = mybir.dt.float32
    f32r = mybir.dt.float32r
    bf16 = mybir.dt.bfloat16
    Act = mybir.ActivationFunctionType

    nc = bacc.Bacc("TRN2", target_bir_lowering=False, debug=False)

    xT = nc.dram_tensor("xT", [C, N], f32r, kind="ExternalInput").ap()
    wqT = nc.dram_tensor("wqT", [C, DV], f32r, kind="ExternalInput").ap()
    wkT = nc.dram_tensor("wkT", [C, DV], f32r, kind="ExternalInput").ap()
    wvT = nc.dram_tensor("wvT", [C, DV], f32r, kind="ExternalInput").ap()
    woT = nc.dram_tensor("woT", [DV, C], f32r, kind="ExternalInput").ap()
    bq = nc.dram_tensor("bq", [DV, 1], f32, kind="ExternalInput").ap()
    bv = nc.dram_tensor("bv", [1, DV], bf16, kind="ExternalInput").ap()
    onesv = nc.dram_tensor("onesv", [1, HPC], bf16, kind="ExternalInput").ap()
    ones64 = nc.dram_tensor("ones64", [1, 64], f32r, kind="ExternalInput").ap()
    y = nc.dram_tensor("y", [N, C], f32, kind="ExternalOutput").ap()
    taps = {}
    if _CACHE.get("debug_taps"):
        taps = {
            "tap_qT": nc.dram_tensor("tap_qT", [P, N], f32, kind="ExternalOutput").ap(),
            "tap_kT": nc.dram_tensor("tap_kT", [P, N], f32, kind="ExternalOutput").ap(),
            "tap_v0": nc.dram_tensor("tap_v0", [P, HPC * VST], f32, kind="ExternalOutput").ap(),
            "tap_e0": nc.dram_tensor("tap_e0", [P, 2 * CHUNK], f32, kind="ExternalOutput").ap(),
            "tap_rz": nc.dram_tensor("tap_rz", [64, CHUNK], f32, kind="ExternalOutput").ap(),
            "tap_at": nc.dram_tensor("tap_at", [P, CHUNK], f32, kind="ExternalOutput").ap(),
            "tap_z": nc.dram_tensor("tap_z", [1, CHUNK], f32, kind="ExternalOutput").ap(),
            "tap_rzrow": nc.dram_tensor("tap_rzrow", [1, CHUNK], f32, kind="ExternalOutput").ap(),
        }

    with tile.TileContext(nc) as tc:
        import contextlib
        with contextlib.ExitStack() as ctx:
            ec = ctx.enter_context
            p_xt = ec(tc.tile_pool(name="p_xt", bufs=KC))
            p_qk = ec(tc.tile_pool(name="p_qk", bufs=2 * NPAIR))
            p_v = ec(tc.tile_pool(name="p_v", bufs=NT))
            p_wqk = ec(tc.tile_pool(name="p_wqk", bufs=3))
            p_wv = ec(tc.tile_pool(name="p_wv", bufs=KC))
            p_wo = ec(tc.tile_pool(name="p_wo", bufs=NPAIR))
            p_e = ec(tc.tile_pool(name="p_e", bufs=6))
            p_at = ec(tc.tile_pool(name="p_at", bufs=NPAIR * NCH))
            p_rz = ec(tc.tile_pool(name="p_rz", bufs=4))
            p_rzbc = ec(tc.tile_pool(name="p_rzbc", bufs=4))
            p_ysb = ec(tc.tile_pool(name="p_ysb", bufs=2))
            p_small = ec(tc.tile_pool(name="p_small", bufs=1))
            p_dram = ec(tc.tile_pool(name="p_dram", bufs=4, space="DRAM"))
            # PSUM: st 2x2 banks + pv 3x1 + aux 1x1 = 8
            p_st = ec(tc.tile_pool(name="p_st", bufs=2, space="PSUM"))
            p_pv = ec(tc.tile_pool(name="p_pv", bufs=3, space="PSUM"))
            p_aux = ec(tc.tile_pool(name="p_aux", bufs=1, space="PSUM"))

            for _rep in range(reps):
                # ---- big x input: quarters, split over both DMA queues ----
                xt_sb = [p_xt.tile([P, N], f32r, tag="xt", name=f"xt{k}")
                         for k in range(KC)]

                def load_xt():
                    for qtr in range(4):
                        for k in range(KC):
                            eng = nc.sync if k % 2 == 0 else nc.scalar
                            c0 = qtr * CHUNK
                            eng.dma_start(
                                xt_sb[k][:, c0:c0 + CHUNK],
                                xT[k * P:(k + 1) * P, c0:c0 + CHUNK])

                # ---- constant/small loads ----
                bq_sb = [p_small.tile([P, 1], f32, name=f"bq{i}")
                         for i in range(NPAIR)]
                for i in range(NPAIR):
                    nc.sync.dma_start(bq_sb[i][:], bq[i * P:(i + 1) * P, :])
                ones64_sb = p_small.tile([65, 64], f32r, name="ones64_sb")
                nc.sync.dma_start(ones64_sb[64:65, :], ones64[:, :])
                bv_bc = p_small.tile([P, DV], bf16, name="bv_bc")
                bv_bcast_ap = bass.AP(tensor=bv.tensor, offset=0,
                                      ap=[[0, P]] + [list(a) for a in bv.ap[1:]])
                nc.scalar.dma_start(bv_bc[:], bv_bcast_ap)

                def load_w_pair(src, pr, label):
                    t = p_wqk.tile([P, KC, P], f32r, tag="wqk",
                                   name=f"{label}{pr}")
                    blk = src[:, pr * P:(pr + 1) * P].rearrange(
                        "(k r) m -> r k m", r=P)
                    nc.sync.dma_start(t[:], blk)
                    return t

                def proj_qk_seg(pr, wq_sb, wk_sb, qTt, kTt, j):
                    q_ps = p_aux.tile([P, CHUNK], f32, tag="aux",
                                      name=f"qps{pr}_{j}")
                    for k in range(KC):
                        nc.tensor.matmul(
                            q_ps[:], wq_sb[:, k, :],
                            xt_sb[k][:, j * CHUNK:(j + 1) * CHUNK],
                            start=(k == 0), stop=(k == KC - 1))
                    nc.vector.tensor_scalar_add(
                        qTt[:, j * CHUNK:(j + 1) * CHUNK], q_ps[:], bq_sb[pr][:])
                    k_ps = p_aux.tile([P, CHUNK], f32, tag="aux",
                                      name=f"kps{pr}_{j}")
                    for k in range(KC):
                        nc.tensor.matmul(
                            k_ps[:], wk_sb[:, k, :],
                            xt_sb[k][:, j * CHUNK:(j + 1) * CHUNK],
                            start=(k == 0), stop=(k == KC - 1))
                    nc.vector.tensor_copy(kTt[:, j * CHUNK:(j + 1) * CHUNK],
                                          k_ps[:])

                qT = [None] * NPAIR
                kT = [None] * NPAIR
                wpair = [None] * NPAIR

                def prep_proj(pr):
                    wpair[pr] = (load_w_pair(wqT, pr, "wq"),
                                 load_w_pair(wkT, pr, "wk"))
                    qT[pr] = p_qk.tile([P, N], bf16, tag="qk", name=f"qT{pr}")
                    kT[pr] = p_qk.tile([P, N], bf16, tag="qk", name=f"kT{pr}")

                def emit_proj(pr):
                    if qT[pr] is None:
                        prep_proj(pr)
                    wq_sb, wk_sb = wpair[pr]
                    for j in range(NCH):
                        proj_qk_seg(pr, wq_sb, wk_sb, qT[pr], kT[pr], j)

                prep_proj(0)
                load_xt()
                emit_proj(0)
                if taps:
                    for nm, srctile in (("tap_qT", qT[0]), ("tap_kT", kT[0])):
                        stg = p_small.tile([P, N], f32, tag="tapqk", name=f"s{nm}")
                        nc.vector.tensor_copy(stg[:], srctile[:])
                        nc.sync.dma_start(taps[nm][:, :], stg[:])

                # ---- v projection ----
                wv_sb = [p_wv.tile([P, DV], f32r, tag="wv", name=f"wv{k}")
                         for k in range(KC)]
                for k in range(KC):
                    nc.scalar.dma_start(wv_sb[k][:], wvT[k * P:(k + 1) * P, :])
                wo_sb = [p_wo.tile([P, C], f32r, tag="wo", name=f"wo{pr}")
                        for pr in range(NPAIR)]
                for pr in range(NPAIR):
                    nc.scalar.dma_start(wo_sb[pr][:], woT[pr * P:(pr + 1) * P, :])

                # v_sb layout: head j=2pr+h at cols 68j..68j+63, ones col at
                # 68j+64 (so the PV stationary [128, 65] puts Z at out row 64)
                v_sb = [p_v.tile([P, HPC * VST], bf16, tag="v", name=f"v{t}")
                        for t in range(NT)]
                onesv_bc = bass.AP(tensor=onesv.tensor, offset=0,
                                   ap=[[0, P], [1, HPC]])
                for t in range(NT):
                    dst = bass.AP(tensor=v_sb[t].tensor,
                                  offset=v_sb[t][:].offset + 64,
                                  ap=[list(v_sb[t][:].ap[0]), [VST, HPC]])
                    nc.scalar.dma_start(dst, onesv_bc)

                def emit_vproj(ts):
                    for t in ts:
                        v_ps = p_aux.tile([P, DV], f32, tag="aux",
                                          name=f"vps{t}")
                        for k in range(KC):
                            nc.tensor.matmul(
                                v_ps[:], xt_sb[k][:, t * P:(t + 1) * P],
                                wv_sb[k][:],
                                start=(k == 0), stop=(k == KC - 1))
                        dst = bass.AP(tensor=v_sb[t].tensor,
                                      offset=v_sb[t][:].offset,
                                      ap=[list(v_sb[t][:].ap[0]),
                                          [VST, HPC], [1, 64]])
                        src = bass.AP(tensor=v_ps.tensor,
                                      offset=v_ps[:].offset,
                                      ap=[list(v_ps[:].ap[0]), [64, HPC], [1, 64]])
                        bvs = bass.AP(tensor=bv_bc.tensor,
                                      offset=bv_bc[:].offset,
                                      ap=[list(bv_bc[:].ap[0]), [64, HPC], [1, 64]])
                        nc.vector.tensor_add(dst, src, bvs)

                emit_vproj(range(NT))
                if taps:
                    stg = p_small.tile([P, HPC * VST], f32, tag="tapv", name="stapv")
                    nc.vector.tensor_copy(stg[:], v_sb[0][:])
                    nc.sync.dma_start(taps["tap_v0"][:, :], stg[:])

                aT = [[None] * NCH for _ in range(NPAIR)]

                def emit_attn(pr, ch):
                    q0 = ch * CHUNK
                    pv = [p_pv.tile([65, CHUNK], f32, tag="pv",
                                    name=f"pv{pr}_{ch}_{h}") for h in range(2)]
                    for t in range(NT):
                        st = p_st.tile([P, 2 * CHUNK], f32, tag="st",
                                       name=f"st{pr}_{ch}_{t}")
                        for h in range(2):
                            hp = h * 64
                            nc.tensor.matmul(
                                st[:, h * CHUNK:(h + 1) * CHUNK],
                                kT[pr][hp:hp + 64, t * P:(t + 1) * P],
                                qT[pr][hp:hp + 64, q0:q0 + CHUNK],
                                start=True, stop=True,
                                tile_position=(hp, 0))
                        e = p_e.tile([P, 2 * CHUNK], bf16, tag="e",
                                     name=f"e{pr}_{ch}_{t}")
                        nc.scalar.activation(e[:], st[:], Act.Exp,
                                             scale=float(SCALE))
                        if taps and pr == 0 and ch == 0 and t == 0:
                            stg = p_small.tile([P, 2 * CHUNK], f32, tag="tape",
                                           name="stape")
                            nc.vector.tensor_copy(stg[:], e[:])
                            nc.sync.dma_start(taps["tap_e0"][:, :], stg[:])
                        for h in range(2):
                            j = 2 * pr + h
                            nc.tensor.matmul(
                                pv[h][:],
                                v_sb[t][:, j * VST:j * VST + 65],
                                e[:, h * CHUNK:(h + 1) * CHUNK],
                                start=(t == 0), stop=(t == NT - 1))
                    # ---- 1/Z off the PSUM Z row (partition-aligned),
                    # DRAM-bounce broadcast, fused evict+scale.  h1's scaled
                    # block is then DMA-moved to partitions 64..127 of a_t
                    # (engines are partition-lockstep; only DMA can move). ----
                    a_t = p_at.tile([P, CHUNK], f32r, tag="at",
                                    name=f"at{pr}_{ch}")
                    for h in range(2):
                        rz_t = p_rz.tile([65, CHUNK], f32r, tag="rz",
                                         name=f"rz{pr}_{ch}_{h}")
                        z_row = rz_t[64:65, :]
                        nc.vector.tensor_copy(z_row, pv[h][64:65, :])
                        # K=1 PE matmul broadcasts Z to partitions 0..63
                        z_ps = p_aux.tile([64, CHUNK], f32, tag="aux",
                                          name=f"zps{pr}_{ch}_{h}")
                        nc.tensor.matmul(z_ps[:], ones64_sb[64:65, :],
                                         z_row, start=True, stop=True,
                                         tile_position=(64, 0))
                        rz_bc = p_rzbc.tile([64, CHUNK], f32, tag="rzbc",
                                            name=f"rzbc{pr}_{ch}_{h}")
                        nc.vector.reciprocal_approx_fast(
                            out=rz_bc[:], in_=z_ps[:])
                        if taps and pr == 0 and ch == 0 and h == 0:
                            nc.sync.dma_start(taps["tap_z"][:, :], z_row.bitcast(f32))
                            nc.sync.dma_start(taps["tap_rzrow"][:, :],
                                              rz_bc[0:1, :])
                        if h == 0:
                            if taps and pr == 0 and ch == 0:
                                nc.sync.dma_start(taps["tap_rz"][:, :], rz_bc[:])
                            nc.vector.tensor_mul(a_t[0:64, :],
                                                 pv[h][0:64, :], rz_bc[:])
                        else:
                            a_h1 = p_rz.tile([64, CHUNK], f32r, tag="ah",
                                             name=f"ah{pr}_{ch}")
                            nc.vector.tensor_mul(a_h1[:],
                                                 pv[h][0:64, :], rz_bc[:])
                            nc.gpsimd.dma_start(a_t[64:128, :], a_h1[:])
                    if taps and pr == 0 and ch == 0:
                        nc.sync.dma_start(taps["tap_at"][:, :],
                                          a_t[:].bitcast(f32))
                    aT[pr][ch] = a_t

                def emit_outproj(ch, last):
                    for mt in range(CHUNK // P):
                        row0 = ch * CHUNK + mt * P
                        y_sb = p_ysb.tile([P, C], f32, tag="ysb",
                                          name=f"ysb{ch}_{mt}")
                        y_ps1 = p_aux.tile([P, CHUNK], f32, tag="aux",
                                           name=f"yp1{ch}_{mt}")
                        for pr in range(NPAIR):
                            lhs = aT[pr][ch][:, mt * P:(mt + 1) * P]
                            nc.tensor.matmul(y_ps1[:], lhs, wo_sb[pr][:, 0:512],
                                             start=(pr == 0),
                                             stop=(pr == NPAIR - 1))
                        if last:
                            nc.scalar.copy(y_sb[:, 0:512], y_ps1[:])
                        else:
                            nc.vector.tensor_copy(y_sb[:, 0:512], y_ps1[:])
                        y_ps2 = p_aux.tile([P, C - 512], f32, tag="aux",
                                           name=f"yp2{ch}_{mt}")
                        for pr in range(NPAIR):
                            lhs = aT[pr][ch][:, mt * P:(mt + 1) * P]
                            nc.tensor.matmul(y_ps2[:], lhs, wo_sb[pr][:, 512:C],
                                             start=(pr == 0),
                                             stop=(pr == NPAIR - 1))
                        if last:
                            nc.scalar.copy(y_sb[:, 512:C], y_ps2[:])
                        else:
                            nc.vector.tensor_copy(y_sb[:, 512:C], y_ps2[:])
                        nc.sync.dma_start(y[row0:row0 + P, :], y_sb[:])

                for pr in range(NPAIR):
                    for ch in range(NCH):
                        emit_attn(pr, ch)
                        if ch == 0 and pr + 1 < NPAIR:
                            emit_proj(pr + 1)
                        if pr == NPAIR - 1:
                            emit_outproj(ch, last=(ch == NCH - 1))

    nc.compile()
    return nc


def _get_nc():
    if "nc" not in _CACHE:
        _CACHE["nc"] = _build()
    return _CACHE["nc"]


def make_in_maps(x, Wq, bq, Wk, Wv, bv, Wo):
    import ml_dtypes
    bf16 = ml_dtypes.bfloat16
    onesv = np.ones((1, HPC), dtype=bf16)
    ones64 = np.ones((1, 64), dtype=np.float32)
    in_maps = []
    for c in range(8):
        b, g = c // 2, c % 2
        sel = slice(g * DV, (g + 1) * DV)
        in_maps.append({
            "xT": np.ascontiguousarray(x[b].T),
            "wqT": np.ascontiguousarray(Wq[sel, :].T),
            "wkT": np.ascontiguousarray(Wk[sel, :].T),
            "wvT": np.ascontiguousarray(Wv[sel, :].T),
            "woT": np.ascontiguousarray(Wo[:, sel].T),
            "bq": np.ascontiguousarray(bq[sel].reshape(DV, 1)),
            "bv": np.ascontiguousarray(bv[sel].reshape(1, DV)).astype(bf16),
            "onesv": onesv,
            "ones64": ones64,
        })
    return in_maps


def kernel(x, Wq, bq, Wk, bk, Wv, bv, Wo, bo, **_unused):
    from concourse.bass_utils import run_bass_kernel_spmd

    x = np.ascontiguousarray(np.asarray(x, dtype=np.float32))
    Wq = np.asarray(Wq, dtype=np.float32)
    Wk = np.asarray(Wk, dtype=np.float32)
    Wv = np.asarray(Wv, dtype=np.float32)
    Wo = np.asarray(Wo, dtype=np.float32)
    bq = np.asarray(bq, dtype=np.float32)
    bv = np.asarray(bv, dtype=np.float32)
    bo = np.asarray(bo, dtype=np.float32)

    in_maps = make_in_maps(x, Wq, bq, Wk, Wv, bv, Wo)
    nc = _get_nc()
    res = run_bass_kernel_spmd(nc, in_maps, core_ids=list(range(8)),
                               trace=bool(_CACHE.get("trace", False)))
    _CACHE["last_result"] = res

    out = np.empty((B, N, C), dtype=np.float32)
    for b in range(B):
        out[b] = res.results[2 * b]["y"] + res.results[2 * b + 1]["y"] + bo
    return out


# revision 25
# speedup vs baseline: 1.4397x; 1.4397x over previous
"""Multi-head attention (B=4, N=2048, C=768, H=12, D=64) on 8 TRN2 NeuronCores.

Sharding: core c handles batch b=c//2 and half the heads (6 heads, g=c%2).
Per core: q/k/v projections for its head slice, S^T-layout attention (nk on
partitions, nq on free), softmax on the Scalar engine only (exp over
[128, 1024] tiles covering both heads of a pair), PV with V as the stationary
operand extended by a ones column (M=65) so the softmax denominator Z
accumulates for free in PSUM row 64, per-(pair,chunk,head) 1/Z via
reciprocal_approx_fast directly on the PSUM Z row, a DRAM-bounce partition
broadcast of 1/Z, fused (evict x 1/Z-scale) of the PV accumulator, and a
partial output projection.  Host sums the two per-batch partials and adds bo.

Relative to the previous version this removes ALL non-exp work from the
Scalar engine (exp is the engine roofline), removes the DVE Z-accumulation
tree and the [1,N] RECIPROCAL (79us profiled) entirely, and drops the
ones-vector Z matmuls from the PE.  qT/kT are bf16 (same PE rate as f32r,
half the eviction/SBUF cost); scores remain ~N(0,64) so bf16 rounding adds
<1% exp error, well inside the 2e-2 gate.

Layout notes: CHUNK=512 (4 chunks); st tiles are [128, 1024] f32 PSUM
(2 banks, h0 cols 0:512, h1 cols 512:1024) written by two tile_position
matmuls and consumed by ONE activation; pv tiles are per-head [65, 512] f32
(1 bank); v_sb rows are [128, 6*68] bf16 with head j at cols 68j..68j+63 and
a ones column at 68j+64 (68-stride keeps 4B alignment).
"""

import numpy as np

B, N, C = 4, 2048, 768
H, D = 12, 64
HPC = 6                 # heads per core
DV = HPC * D            # 384
P = 128
KC = C // P             # 6 contraction chunks for projections
NPAIR = 3               # head-pairs per core
NT = N // P             # 16 nk tiles
CHUNK = 512
NCH = N // CHUNK        # 4 chunks
VST = 68                # v_sb per-head stride (64 data + 1 ones + 3 pad)
SCALE = 1.0 / np.sqrt(D)

_CACHE = {}


def _build(reps=1):
    import warnings
    warnings.filterwarnings("ignore")
    import concourse.bass as bass
    import concourse.bacc as bacc
    import concourse.mybir as mybir
    from concourse import tile

    f32 = mybir.dt.float32
    f32r = mybir.dt.float32r
    bf16 = mybir.dt.bfloat16
    Act = mybir.ActivationFunctionType

    nc = bacc.Bacc("TRN2", target_bir_lowering=False, debug=False)

    xT = nc.dram_tensor("xT", [C, N], f32r, kind="ExternalInput").ap()
    wqT = nc.dram_tensor("wqT", [C, DV], f32r, kind="ExternalInput").ap()
    wkT = nc.dram_tensor("wkT", [C, DV], f32r, kind="ExternalInput").ap()
    wvT = nc.dram_tensor("wvT", [C, DV], f32r, kind="ExternalInput").ap()
    woT = nc.dram_tensor("woT", [DV, C], f32r, kind="ExternalInput").ap()
    bq = nc.dram_tensor("bq", [DV, 1], f32, kind="ExternalInput").ap()
    bv = nc.dram_tensor("bv", [1, DV], bf16, kind="ExternalInput").ap()
    y = nc.dram_tensor("y", [N, C], f32, kind="ExternalOutput").ap()
    taps = {}
    if _CACHE.get("debug_taps"):
        taps = {
            "tap_qT": nc.dram_tensor("tap_qT", [P, N], f32, kind="ExternalOutput").ap(),
            "tap_kT": nc.dram_tensor("tap_kT", [P, N], f32, kind="ExternalOutput").ap(),
            "tap_v0": nc.dram_tensor("tap_v0", [P, HPC * VST], f32, kind="ExternalOutput").ap(),
            "tap_e0": nc.dram_tensor("tap_e0", [P, 2 * CHUNK], f32, kind="ExternalOutput").ap(),
            "tap_rz": nc.dram_tensor("tap_rz", [64, CHUNK], f32, kind="ExternalOutput").ap(),
            "tap_at": nc.dram_tensor("tap_at", [P, CHUNK], f32, kind="ExternalOutput").ap(),
            "tap_z": nc.dram_tensor("tap_z", [1, CHUNK], f32, kind="ExternalOutput").ap(),
            "tap_rzrow": nc.dram_tensor("tap_rzrow", [1, CHUNK], f32, kind="ExternalOutput").ap(),
        }

    with tile.TileContext(nc) as tc:
        import contextlib
        with contextlib.ExitStack() as ctx:
            ec = ctx.enter_context
            p_xt = ec(tc.tile_pool(name="p_xt", bufs=KC))
            p_qk = ec(tc.tile_pool(name="p_qk", bufs=2 * NPAIR))
            p_v = ec(tc.tile_pool(name="p_v", bufs=NT))
            p_wqk = ec(tc.tile_pool(name="p_wqk", bufs=3))
            p_wv = ec(tc.tile_pool(name="p_wv", bufs=KC))
            p_wo = ec(tc.tile_pool(name="p_wo", bufs=NPAIR))
            p_e = ec(tc.tile_pool(name="p_e", bufs=6))
            p_at = ec(tc.tile_pool(name="p_at", bufs=NPAIR * NCH))
            p_rz = ec(tc.tile_pool(name="p_rz", bufs=4))
            p_rzbc = ec(tc.tile_pool(name="p_rzbc", bufs=4))
            p_ysb = ec(tc.tile_pool(name="p_ysb", bufs=2))
            p_small = ec(tc.tile_pool(name="p_small", bufs=1))
            p_dram = ec(tc.tile_pool(name="p_dram", bufs=4, space="DRAM"))
            # PSUM: st 2x2 banks + pv 3x1 + aux 1x1 = 8
            p_st = ec(tc.tile_pool(name="p_st", bufs=2, space="PSUM"))
            p_pv = ec(tc.tile_pool(name="p_pv", bufs=3, space="PSUM"))
            p_aux = ec(tc.tile_pool(name="p_aux", bufs=1, space="PSUM"))

            for _rep in range(reps):
                # ---- big x input: quarters, split over both DMA queues ----
                xt_sb = [p_xt.tile([P, N], f32r, tag="xt", name=f"xt{k}")
                         for k in range(KC)]

                def load_xt():
                    for hlf in range(2):
                        for k in range(KC):
                            eng = nc.sync if k % 2 == 0 else nc.scalar
                            c0 = hlf * (N // 2)
                            eng.dma_start(
                                xt_sb[k][:, c0:c0 + N // 2],
                                xT[k * P:(k + 1) * P, c0:c0 + N // 2])

                # ---- constant/small loads ----
                bq_sb = [p_small.tile([P, 1], f32, name=f"bq{i}")
                         for i in range(NPAIR)]
                for i in range(NPAIR):
                    nc.sync.dma_start(bq_sb[i][:], bq[i * P:(i + 1) * P, :])
                ones64_sb = p_small.tile([65, 64], f32, name="ones64_sb")
                nc.vector.memset(ones64_sb[64:65, :], 1.0)
                bv_bc = p_small.tile([P, DV], bf16, name="bv_bc")
                bv_bcast_ap = bass.AP(tensor=bv.tensor, offset=0,
                                      ap=[[0, P]] + [list(a) for a in bv.ap[1:]])
                nc.scalar.dma_start(bv_bc[:], bv_bcast_ap)

                def load_w_pair(src, pr, label):
                    t = p_wqk.tile([P, KC, P], f32r, tag="wqk",
                                   name=f"{label}{pr}")
                    blk = src[:, pr * P:(pr + 1) * P].rearrange(
                        "(k r) m -> r k m", r=P)
                    nc.sync.dma_start(t[:], blk)
                    return t

                def proj_qk_seg(pr, wq_sb, wk_sb, qTt, kTt, j):
                    q_ps = p_aux.tile([P, CHUNK], f32, tag="aux",
                                      name=f"qps{pr}_{j}")
                    for k in range(KC):
                        nc.tensor.matmul(
                            q_ps[:], wq_sb[:, k, :],
                            xt_sb[k][:, j * CHUNK:(j + 1) * CHUNK],
                            start=(k == 0), stop=(k == KC - 1))
                    nc.vector.tensor_scalar_add(
                        qTt[:, j * CHUNK:(j + 1) * CHUNK], q_ps[:], bq_sb[pr][:])
                    k_ps = p_aux.tile([P, CHUNK], f32, tag="aux",
                                      name=f"kps{pr}_{j}")
                    for k in range(KC):
                        nc.tensor.matmul(
                            k_ps[:], wk_sb[:, k, :],
                            xt_sb[k][:, j * CHUNK:(j + 1) * CHUNK],
                            start=(k == 0), stop=(k == KC - 1))
                    nc.vector.tensor_copy(kTt[:, j * CHUNK:(j + 1) * CHUNK],
                                          k_ps[:])

                qT = [None] * NPAIR
                kT = [None] * NPAIR
                wpair = [None] * NPAIR

                def prep_proj(pr):
                    wpair[pr] = (load_w_pair(wqT, pr, "wq"),
                                 load_w_pair(wkT, pr, "wk"))
                    qT[pr] = p_qk.tile([P, N], bf16, tag="qk", name=f"qT{pr}")
                    kT[pr] = p_qk.tile([P, N], bf16, tag="qk", name=f"kT{pr}")

                def emit_proj(pr):
                    if qT[pr] is None:
                        prep_proj(pr)
                    wq_sb, wk_sb = wpair[pr]
                    for j in range(NCH):
                        proj_qk_seg(pr, wq_sb, wk_sb, qT[pr], kT[pr], j)

                prep_proj(0)
                load_xt()
                emit_proj(0)
                if taps:
                    for nm, srctile in (("tap_qT", qT[0]), ("tap_kT", kT[0])):
                        stg = p_small.tile([P, N], f32, tag="tapqk", name=f"s{nm}")
                        nc.vector.tensor_copy(stg[:], srctile[:])
                        nc.sync.dma_start(taps[nm][:, :], stg[:])

                # ---- v projection ----
                wv_sb = [p_wv.tile([P, DV], f32r, tag="wv", name=f"wv{k}")
                         for k in range(KC)]
                for k in range(KC):
                    nc.scalar.dma_start(wv_sb[k][:], wvT[k * P:(k + 1) * P, :])
                wo_sb = [p_wo.tile([P, C], f32r, tag="wo", name=f"wo{pr}")
                        for pr in range(NPAIR)]
                for pr in range(NPAIR):
                    nc.scalar.dma_start(wo_sb[pr][:], woT[pr * P:(pr + 1) * P, :])

                # v_sb layout: head j=2pr+h at cols 68j..68j+63, ones col at
                # 68j+64 (so the PV stationary [128, 65] puts Z at out row 64)
                v_sb = [p_v.tile([P, HPC * VST], bf16, tag="v", name=f"v{t}")
                        for t in range(NT)]
                for t in range(NT):
                    dst = bass.AP(tensor=v_sb[t].tensor,
                                  offset=v_sb[t][:].offset + 64,
                                  ap=[list(v_sb[t][:].ap[0]), [VST, HPC]])
                    nc.vector.memset(dst, 1.0)

                def emit_vproj(ts):
                    for t in ts:
                        v_ps = p_aux.tile([P, DV], f32, tag="aux",
                                          name=f"vps{t}")
                        for k in range(KC):
                            nc.tensor.matmul(
                                v_ps[:], xt_sb[k][:, t * P:(t + 1) * P],
                                wv_sb[k][:],
                                start=(k == 0), stop=(k == KC - 1))
                        dst = bass.AP(tensor=v_sb[t].tensor,
                                      offset=v_sb[t][:].offset,
                                      ap=[list(v_sb[t][:].ap[0]),
                                          [VST, HPC], [1, 64]])
                        src = bass.AP(tensor=v_ps.tensor,
                                      offset=v_ps[:].offset,
                                      ap=[list(v_ps[:].ap[0]), [64, HPC], [1, 64]])
                        bvs = bass.AP(tensor=bv_bc.tensor,
                                      offset=bv_bc[:].offset,
                                      ap=[list(bv_bc[:].ap[0]), [64, HPC], [1, 64]])
                        nc.vector.tensor_add(dst, src, bvs)

                emit_vproj(range(NT))
                if taps:
                    stg = p_small.tile([P, HPC * VST], f32, tag="tapv", name="stapv")
                    nc.vector.tensor_copy(stg[:], v_sb[0][:])
                    nc.sync.dma_start(taps["tap_v0"][:, :], stg[:])

                aT = [[None] * NCH for _ in range(NPAIR)]

                def emit_attn(pr, ch):
                    q0 = ch * CHUNK
                    pv = [p_pv.tile([65, CHUNK], f32, tag="pv",
                                    name=f"pv{pr}_{ch}_{h}") for h in range(2)]
                    for t in range(NT):
                        st = p_st.tile([P, 2 * CHUNK], f32, tag="st",
                                       name=f"st{pr}_{ch}_{t}")
                        for h in range(2):
                            hp = h * 64
                            nc.tensor.matmul(
                                st[:, h * CHUNK:(h + 1) * CHUNK],
                                kT[pr][hp:hp + 64, t * P:(t + 1) * P],
                                qT[pr][hp:hp + 64, q0:q0 + CHUNK],
                                start=True, stop=True,
                                tile_position=(hp, 0))
                        e = p_e.tile([P, 2 * CHUNK], bf16, tag="e",
                                     name=f"e{pr}_{ch}_{t}")
                        nc.scalar.activation(e[:], st[:], Act.Exp,
                                             scale=float(SCALE))
                        if taps and pr == 0 and ch == 0 and t == 0:
                            stg = p_small.tile([P, 2 * CHUNK], f32, tag="tape",
                                           name="stape")
                            nc.vector.tensor_copy(stg[:], e[:])
                            nc.sync.dma_start(taps["tap_e0"][:, :], stg[:])
                        for h in range(2):
                            j = 2 * pr + h
                            nc.tensor.matmul(
                                pv[h][:],
                                v_sb[t][:, j * VST:j * VST + 65],
                                e[:, h * CHUNK:(h + 1) * CHUNK],
                                start=(t == 0), stop=(t == NT - 1))
                    # ---- 1/Z off the PSUM Z row (partition-aligned),
                    # DRAM-bounce broadcast, fused evict+scale.  h1's scaled
                    # block is then DMA-moved to partitions 64..127 of a_t
                    # (engines are partition-lockstep; only DMA can move). ----
                    a_t = p_at.tile([P, CHUNK], f32r, tag="at",
                                    name=f"at{pr}_{ch}")
                    for h in range(2):
                        rz_t = p_rz.tile([65, CHUNK], f32r, tag="rz",
                                         name=f"rz{pr}_{ch}_{h}")
                        z_row = rz_t[64:65, :]
                        nc.vector.tensor_copy(z_row, pv[h][64:65, :])
                        # K=1 PE matmul broadcasts Z to partitions 0..63
                        z_ps = p_aux.tile([64, CHUNK], f32, tag="aux",
                                          name=f"zps{pr}_{ch}_{h}")
                        nc.tensor.matmul(z_ps[:],
                                         ones64_sb[64:65, :].bitcast(f32r),
                                         z_row, start=True, stop=True,
                                         tile_position=(64, 0))
                        rz_bc = p_rzbc.tile([64, CHUNK], f32, tag="rzbc",
                                            name=f"rzbc{pr}_{ch}_{h}")
                        nc.vector.reciprocal_approx_fast(
                            out=rz_bc[:], in_=z_ps[:])
                        if taps and pr == 0 and ch == 0 and h == 0:
                            nc.sync.dma_start(taps["tap_z"][:, :], z_row.bitcast(f32))
                            nc.sync.dma_start(taps["tap_rzrow"][:, :],
                                              rz_bc[0:1, :])
                        if h == 0:
                            if taps and pr == 0 and ch == 0:
                                nc.sync.dma_start(taps["tap_rz"][:, :], rz_bc[:])
                            nc.vector.tensor_mul(a_t[0:64, :],
                                                 pv[h][0:64, :], rz_bc[:])
                        else:
                            a_h1 = p_rz.tile([64, CHUNK], f32r, tag="ah",
                                             name=f"ah{pr}_{ch}")
                            nc.vector.tensor_mul(a_h1[:],
                                                 pv[h][0:64, :], rz_bc[:])
                            nc.gpsimd.dma_start(a_t[64:128, :], a_h1[:])
                    if taps and pr == 0 and ch == 0:
                        nc.sync.dma_start(taps["tap_at"][:, :],
                                          a_t[:].bitcast(f32))
                    aT[pr][ch] = a_t

                def emit_outproj(ch, last):
                    for mt in range(CHUNK // P):
                        row0 = ch * CHUNK + mt * P
                        y_sb = p_ysb.tile([P, C], f32, tag="ysb",
                                          name=f"ysb{ch}_{mt}")
                        y_ps1 = p_aux.tile([P, CHUNK], f32, tag="aux",
                                           name=f"yp1{ch}_{mt}")
                        for pr in range(NPAIR):
                            lhs = aT[pr][ch][:, mt * P:(mt + 1) * P]
                            nc.tensor.matmul(y_ps1[:], lhs, wo_sb[pr][:, 0:512],
                                             start=(pr == 0),
                                             stop=(pr == NPAIR - 1))
                        if last:
                            nc.scalar.copy(y_sb[:, 0:512], y_ps1[:])
                        else:
                            nc.vector.tensor_copy(y_sb[:, 0:512], y_ps1[:])
                        y_ps2 = p_aux.tile([P, C - 512], f32, tag="aux",
                                           name=f"yp2{ch}_{mt}")
                        for pr in range(NPAIR):
                            lhs = aT[pr][ch][:, mt * P:(mt + 1) * P]
                            nc.tensor.matmul(y_ps2[:], lhs, wo_sb[pr][:, 512:C],
                                             start=(pr == 0),
                                             stop=(pr == NPAIR - 1))
                        if last:
                            nc.scalar.copy(y_sb[:, 512:C], y_ps2[:])
                        else:
                            nc.vector.tensor_copy(y_sb[:, 512:C], y_ps2[:])
                        nc.sync.dma_start(y[row0:row0 + P, :], y_sb[:])

                for pr in range(NPAIR):
                    for ch in range(NCH):
                        emit_attn(pr, ch)
                        if ch == 0 and pr + 1 < NPAIR:
                            emit_proj(pr + 1)
                        if pr == NPAIR - 1:
                            emit_outproj(ch, last=(ch == NCH - 1))

    nc.compile()
    return nc


def _get_nc():
    if "nc" not in _CACHE:
        _CACHE["nc"] = _build()
    return _CACHE["nc"]


def make_in_maps(x, Wq, bq, Wk, Wv, bv, Wo):
    import ml_dtypes
    bf16 = ml_dtypes.bfloat16
    in_maps = []
    for c in range(8):
        b, g = c // 2, c % 2
        sel = slice(g * DV, (g + 1) * DV)
        in_maps.append({
            "xT": np.ascontiguousarray(x[b].T),
            "wqT": np.ascontiguousarray(Wq[sel, :].T),
            "wkT": np.ascontiguousarray(Wk[sel, :].T),
            "wvT": np.ascontiguousarray(Wv[sel, :].T),
            "woT": np.ascontiguousarray(Wo[:, sel].T),
            "bq": np.ascontiguousarray(bq[sel].reshape(DV, 1)),
            "bv": np.ascontiguousarray(bv[sel].reshape(1, DV)).astype(bf16),
        })
    return in_maps


def kernel(x, Wq, bq, Wk, bk, Wv, bv, Wo, bo, **_unused):
    from concourse.bass_utils import run_bass_kernel_spmd

    x = np.ascontiguousarray(np.asarray(x, dtype=np.float32))
    Wq = np.asarray(Wq, dtype=np.float32)
    Wk = np.asarray(Wk, dtype=np.float32)
    Wv = np.asarray(Wv, dtype=np.float32)
    Wo = np.asarray(Wo, dtype=np.float32)
    bq = np.asarray(bq, dtype=np.float32)
    bv = np.asarray(bv, dtype=np.float32)
    bo = np.asarray(bo, dtype=np.float32)

    in_maps = make_in_maps(x, Wq, bq, Wk, Wv, bv, Wo)
    nc = _get_nc()
    res = run_bass_kernel_spmd(nc, in_maps, core_ids=list(range(8)),
                               trace=bool(_CACHE.get("trace", False)))
    _CACHE["last_result"] = res

    out = np.empty((B, N, C), dtype=np.float32)
    for b in range(B):
        out[b] = res.results[2 * b]["y"] + res.results[2 * b + 1]["y"] + bo
    return out


# revision 28
# speedup vs baseline: 1.4899x; 1.0348x over previous
"""Multi-head attention (B=4, N=2048, C=768, H=12, D=64) on 8 TRN2 NeuronCores.

Sharding: core c handles batch b=c//2 and half the heads (6 heads, g=c%2).
Per core: q/k/v projections for its head slice, S^T-layout attention (nk on
partitions, nq on free), softmax on the Scalar engine only (exp over
[128, 1024] tiles covering both heads of a pair), PV with V as the stationary
operand extended by a ones column (M=65) so the softmax denominator Z
accumulates for free in PSUM row 64, per-(pair,chunk,head) 1/Z via
reciprocal_approx_fast directly on the PSUM Z row, a DRAM-bounce partition
broadcast of 1/Z, fused (evict x 1/Z-scale) of the PV accumulator, and a
partial output projection.  Host sums the two per-batch partials and adds bo.

Relative to the previous version this removes ALL non-exp work from the
Scalar engine (exp is the engine roofline), removes the DVE Z-accumulation
tree and the [1,N] RECIPROCAL (79us profiled) entirely, and drops the
ones-vector Z matmuls from the PE.  qT/kT are bf16 (same PE rate as f32r,
half the eviction/SBUF cost); scores remain ~N(0,64) so bf16 rounding adds
<1% exp error, well inside the 2e-2 gate.

Layout notes: CHUNK=512 (4 chunks); st tiles are [128, 1024] f32 PSUM
(2 banks, h0 cols 0:512, h1 cols 512:1024) written by two tile_position
matmuls and consumed by ONE activation; pv tiles are per-head [65, 512] f32
(1 bank); v_sb rows are [128, 6*68] bf16 with head j at cols 68j..68j+63 and
a ones column at 68j+64 (68-stride keeps 4B alignment).
"""

import numpy as np

B, N, C = 4, 2048, 768
H, D = 12, 64
HPC = 6                 # heads per core
DV = HPC * D            # 384
P = 128
KC = C // P             # 6 contraction chunks for projections
NPAIR = 3               # head-pairs per core
NT = N // P             # 16 nk tiles
CHUNK = 512
NCH = N // CHUNK        # 4 chunks
VST = 68                # v_sb per-head stride (64 data + 1 ones + 3 pad)
SCALE = 1.0 / np.sqrt(D)

_CACHE = {}


def _build(reps=1):
    import warnings
    warnings.filterwarnings("ignore")
    import concourse.bass as bass
    import concourse.bacc as bacc
    import concourse.mybir as mybir
    from concourse import tile

    f32 = mybir.dt.float32
    f32r = mybir.dt.float32r
    bf16 = mybir.dt.bfloat16
    Act = mybir.ActivationFunctionType

    nc = bacc.Bacc("TRN2", target_bir_lowering=False, debug=False)

    xT = nc.dram_tensor("xT", [C, N], f32r, kind="ExternalInput").ap()
    wqT = nc.dram_tensor("wqT", [C, DV], f32r, kind="ExternalInput").ap()
    wkT = nc.dram_tensor("wkT", [C, DV], f32r, kind="ExternalInput").ap()
    wvT = nc.dram_tensor("wvT", [C, DV], f32r, kind="ExternalInput").ap()
    woT = nc.dram_tensor("woT", [DV, C], f32r, kind="ExternalInput").ap()
    bq = nc.dram_tensor("bq", [DV, 1], f32, kind="ExternalInput").ap()
    bv = nc.dram_tensor("bv", [1, DV], bf16, kind="ExternalInput").ap()
    y = nc.dram_tensor("y", [N, C], f32, kind="ExternalOutput").ap()
    taps = {}
    if _CACHE.get("debug_taps"):
        taps = {
            "tap_qT": nc.dram_tensor("tap_qT", [P, N], f32, kind="ExternalOutput").ap(),
            "tap_kT": nc.dram_tensor("tap_kT", [P, N], f32, kind="ExternalOutput").ap(),
            "tap_v0": nc.dram_tensor("tap_v0", [P, HPC * VST], f32, kind="ExternalOutput").ap(),
            "tap_e0": nc.dram_tensor("tap_e0", [P, 2 * CHUNK], f32, kind="ExternalOutput").ap(),
            "tap_rz": nc.dram_tensor("tap_rz", [64, CHUNK], f32, kind="ExternalOutput").ap(),
            "tap_at": nc.dram_tensor("tap_at", [P, CHUNK], f32, kind="ExternalOutput").ap(),
            "tap_z": nc.dram_tensor("tap_z", [1, CHUNK], f32, kind="ExternalOutput").ap(),
            "tap_rzrow": nc.dram_tensor("tap_rzrow", [1, CHUNK], f32, kind="ExternalOutput").ap(),
        }

    with tile.TileContext(nc) as tc:
        import contextlib
        with contextlib.ExitStack() as ctx:
            ec = ctx.enter_context
            p_xt = ec(tc.tile_pool(name="p_xt", bufs=KC))
            p_qk = ec(tc.tile_pool(name="p_qk", bufs=2 * NPAIR))
            p_v = ec(tc.tile_pool(name="p_v", bufs=NT))
            p_wqk = ec(tc.tile_pool(name="p_wqk", bufs=3))
            p_wv = ec(tc.tile_pool(name="p_wv", bufs=KC))
            p_wo = ec(tc.tile_pool(name="p_wo", bufs=NPAIR))
            p_e = ec(tc.tile_pool(name="p_e", bufs=6))
            p_at = ec(tc.tile_pool(name="p_at", bufs=NPAIR * NCH))
            p_rz = ec(tc.tile_pool(name="p_rz", bufs=4))
            p_rzbc = ec(tc.tile_pool(name="p_rzbc", bufs=4))
            p_ysb = ec(tc.tile_pool(name="p_ysb", bufs=2))
            p_small = ec(tc.tile_pool(name="p_small", bufs=1))
            p_dram = ec(tc.tile_pool(name="p_dram", bufs=4, space="DRAM"))
            # PSUM: st 2x2 banks + pv 3x1 + aux 1x1 = 8
            p_st = ec(tc.tile_pool(name="p_st", bufs=2, space="PSUM"))
            p_pv = ec(tc.tile_pool(name="p_pv", bufs=3, space="PSUM"))
            p_aux = ec(tc.tile_pool(name="p_aux", bufs=1, space="PSUM"))

            for _rep in range(reps):
                # ---- big x input: quarters, split over both DMA queues ----
                xt_sb = [p_xt.tile([P, N], f32r, tag="xt", name=f"xt{k}")
                         for k in range(KC)]

                def load_xt():
                    for hlf in range(2):
                        for k in range(KC):
                            eng = nc.sync if k % 2 == 0 else nc.scalar
                            c0 = hlf * (N // 2)
                            eng.dma_start(
                                xt_sb[k][:, c0:c0 + N // 2],
                                xT[k * P:(k + 1) * P, c0:c0 + N // 2])

                # ---- constant/small loads ----
                bq_sb = [p_small.tile([P, 1], f32, name=f"bq{i}")
                         for i in range(NPAIR)]
                for i in range(NPAIR):
                    nc.sync.dma_start(bq_sb[i][:], bq[i * P:(i + 1) * P, :])
                ones64_sb = p_small.tile([65, 64], f32, name="ones64_sb")
                nc.vector.memset(ones64_sb[64:65, :], 1.0)
                bv_bc = p_small.tile([P, DV], bf16, name="bv_bc")
                bv_bcast_ap = bass.AP(tensor=bv.tensor, offset=0,
                                      ap=[[0, P]] + [list(a) for a in bv.ap[1:]])
                nc.scalar.dma_start(bv_bc[:], bv_bcast_ap)

                def load_w_pair(src, pr, label):
                    t = p_wqk.tile([P, KC, P], f32r, tag="wqk",
                                   name=f"{label}{pr}")
                    blk = src[:, pr * P:(pr + 1) * P].rearrange(
                        "(k r) m -> r k m", r=P)
                    nc.sync.dma_start(t[:], blk)
                    return t

                def proj_qk_seg(pr, wq_sb, wk_sb, qTt, kTt, j):
                    q_ps = p_aux.tile([P, CHUNK], f32, tag="aux",
                                      name=f"qps{pr}_{j}")
                    for k in range(KC):
                        nc.tensor.matmul(
                            q_ps[:], wq_sb[:, k, :],
                            xt_sb[k][:, j * CHUNK:(j + 1) * CHUNK],
                            start=(k == 0), stop=(k == KC - 1))
                    nc.vector.tensor_scalar_add(
                        qTt[:, j * CHUNK:(j + 1) * CHUNK], q_ps[:], bq_sb[pr][:])
                    k_ps = p_aux.tile([P, CHUNK], f32, tag="aux",
                                      name=f"kps{pr}_{j}")
                    for k in range(KC):
                        nc.tensor.matmul(
                            k_ps[:], wk_sb[:, k, :],
                            xt_sb[k][:, j * CHUNK:(j + 1) * CHUNK],
                            start=(k == 0), stop=(k == KC - 1))
                    nc.vector.tensor_copy(kTt[:, j * CHUNK:(j + 1) * CHUNK],
                                          k_ps[:])

                qT = [None] * NPAIR
                kT = [None] * NPAIR
                wpair = [None] * NPAIR

                def prep_proj(pr):
                    wpair[pr] = (load_w_pair(wqT, pr, "wq"),
                                 load_w_pair(wkT, pr, "wk"))
                    qT[pr] = p_qk.tile([P, N], bf16, tag="qk", name=f"qT{pr}")
                    kT[pr] = p_qk.tile([P, N], bf16, tag="qk", name=f"kT{pr}")

                def emit_proj(pr):
                    if qT[pr] is None:
                        prep_proj(pr)
                    wq_sb, wk_sb = wpair[pr]
                    for j in range(NCH):
                        proj_qk_seg(pr, wq_sb, wk_sb, qT[pr], kT[pr], j)

                prep_proj(0)
                load_xt()
                emit_proj(0)
                if taps:
                    for nm, srctile in (("tap_qT", qT[0]), ("tap_kT", kT[0])):
                        stg = p_small.tile([P, N], f32, tag="tapqk", name=f"s{nm}")
                        nc.vector.tensor_copy(stg[:], srctile[:])
                        nc.sync.dma_start(taps[nm][:, :], stg[:])

                # ---- v projection ----
                wv_sb = [p_wv.tile([P, DV], f32r, tag="wv", name=f"wv{k}")
                         for k in range(KC)]
                for k in range(KC):
                    nc.scalar.dma_start(wv_sb[k][:], wvT[k * P:(k + 1) * P, :])
                wo_sb = [p_wo.tile([P, C], f32r, tag="wo", name=f"wo{pr}")
                        for pr in range(NPAIR)]
                for pr in range(NPAIR):
                    nc.scalar.dma_start(wo_sb[pr][:], woT[pr * P:(pr + 1) * P, :])

                # v_sb layout: head j=2pr+h at cols 68j..68j+63, ones col at
                # 68j+64 (so the PV stationary [128, 65] puts Z at out row 64)
                v_sb = [p_v.tile([P, HPC * VST], bf16, tag="v", name=f"v{t}")
                        for t in range(NT)]
                for t in range(NT):
                    dst = bass.AP(tensor=v_sb[t].tensor,
                                  offset=v_sb[t][:].offset + 64,
                                  ap=[list(v_sb[t][:].ap[0]), [VST, HPC]])
                    nc.vector.memset(dst, 1.0)

                def emit_vproj(ts):
                    for t in ts:
                        v_ps = p_aux.tile([P, DV], f32, tag="aux",
                                          name=f"vps{t}")
                        for k in range(KC):
                            nc.tensor.matmul(
                                v_ps[:], xt_sb[k][:, t * P:(t + 1) * P],
                                wv_sb[k][:],
                                start=(k == 0), stop=(k == KC - 1))
                        dst = bass.AP(tensor=v_sb[t].tensor,
                                      offset=v_sb[t][:].offset,
                                      ap=[list(v_sb[t][:].ap[0]),
                                          [VST, HPC], [1, 64]])
                        src = bass.AP(tensor=v_ps.tensor,
                                      offset=v_ps[:].offset,
                                      ap=[list(v_ps[:].ap[0]), [64, HPC], [1, 64]])
                        bvs = bass.AP(tensor=bv_bc.tensor,
                                      offset=bv_bc[:].offset,
                                      ap=[list(bv_bc[:].ap[0]), [64, HPC], [1, 64]])
                        nc.vector.tensor_add(dst, src, bvs)

                emit_vproj(range(NT))
                if taps:
                    stg = p_small.tile([P, HPC * VST], f32, tag="tapv", name="stapv")
                    nc.vector.tensor_copy(stg[:], v_sb[0][:])
                    nc.sync.dma_start(taps["tap_v0"][:, :], stg[:])

                aT = [[None] * NCH for _ in range(NPAIR)]

                def emit_attn(pr, ch):
                    q0 = ch * CHUNK
                    pv = [p_pv.tile([65, CHUNK], f32, tag="pv",
                                    name=f"pv{pr}_{ch}_{h}") for h in range(2)]
                    es = [None] * NT

                    def emit_pv(t):
                        for h in range(2):
                            j = 2 * pr + h
                            nc.tensor.matmul(
                                pv[h][:],
                                v_sb[t][:, j * VST:j * VST + 65],
                                es[t][:, h * CHUNK:(h + 1) * CHUNK],
                                start=(t == 0), stop=(t == NT - 1))

                    for t in range(NT):
                        st = p_st.tile([P, 2 * CHUNK], f32, tag="st",
                                       name=f"st{pr}_{ch}_{t}")
                        for h in range(2):
                            hp = h * 64
                            nc.tensor.matmul(
                                st[:, h * CHUNK:(h + 1) * CHUNK],
                                kT[pr][hp:hp + 64, t * P:(t + 1) * P],
                                qT[pr][hp:hp + 64, q0:q0 + CHUNK],
                                start=True, stop=True,
                                tile_position=(hp, 0))
                        es[t] = p_e.tile([P, 2 * CHUNK], bf16, tag="e",
                                         name=f"e{pr}_{ch}_{t}")
                        nc.scalar.activation(es[t][:], st[:], Act.Exp,
                                             scale=float(SCALE))
                        if taps and pr == 0 and ch == 0 and t == 0:
                            stg = p_small.tile([P, 2 * CHUNK], f32, tag="tape",
                                           name="stape")
                            nc.vector.tensor_copy(stg[:], es[t][:])
                            nc.sync.dma_start(taps["tap_e0"][:, :], stg[:])
                        # PV deferred one tile: keeps the PE FIFO from
                        # head-of-line blocking on the exp it just queued
                        if t > 0:
                            emit_pv(t - 1)
                    emit_pv(NT - 1)
                    # ---- 1/Z off the PSUM Z row (partition-aligned),
                    # DRAM-bounce broadcast, fused evict+scale.  h1's scaled
                    # block is then DMA-moved to partitions 64..127 of a_t
                    # (engines are partition-lockstep; only DMA can move). ----
                    a_t = p_at.tile([P, CHUNK], f32r, tag="at",
                                    name=f"at{pr}_{ch}")
                    for h in range(2):
                        rz_t = p_rz.tile([65, CHUNK], f32r, tag="rz",
                                         name=f"rz{pr}_{ch}_{h}")
                        z_row = rz_t[64:65, :]
                        nc.vector.tensor_copy(z_row, pv[h][64:65, :])
                        # K=1 PE matmul broadcasts Z to partitions 0..63
                        z_ps = p_aux.tile([64, CHUNK], f32, tag="aux",
                                          name=f"zps{pr}_{ch}_{h}")
                        nc.tensor.matmul(z_ps[:],
                                         ones64_sb[64:65, :].bitcast(f32r),
                                         z_row, start=True, stop=True,
                                         tile_position=(64, 0))
                        rz_bc = p_rzbc.tile([64, CHUNK], f32, tag="rzbc",
                                            name=f"rzbc{pr}_{ch}_{h}")
                        nc.vector.reciprocal_approx_fast(
                            out=rz_bc[:], in_=z_ps[:])
                        if taps and pr == 0 and ch == 0 and h == 0:
                            nc.sync.dma_start(taps["tap_z"][:, :], z_row.bitcast(f32))
                            nc.sync.dma_start(taps["tap_rzrow"][:, :],
                                              rz_bc[0:1, :])
                        if h == 0:
                            if taps and pr == 0 and ch == 0:
                                nc.sync.dma_start(taps["tap_rz"][:, :], rz_bc[:])
                            nc.vector.tensor_mul(a_t[0:64, :],
                                                 pv[h][0:64, :], rz_bc[:])
                        else:
                            a_h1 = p_rz.tile([64, CHUNK], f32r, tag="ah",
                                             name=f"ah{pr}_{ch}")
                            nc.vector.tensor_mul(a_h1[:],
                                                 pv[h][0:64, :], rz_bc[:])
                            nc.gpsimd.dma_start(a_t[64:128, :], a_h1[:])
                    if taps and pr == 0 and ch == 0:
                        nc.sync.dma_start(taps["tap_at"][:, :],
                                          a_t[:].bitcast(f32))
                    aT[pr][ch] = a_t

                def emit_outproj(ch, last):
                    for mt in range(CHUNK // P):
                        row0 = ch * CHUNK + mt * P
                        y_sb = p_ysb.tile([P, C], f32, tag="ysb",
                                          name=f"ysb{ch}_{mt}")
                        y_ps1 = p_aux.tile([P, CHUNK], f32, tag="aux",
                                           name=f"yp1{ch}_{mt}")
                        for pr in range(NPAIR):
                            lhs = aT[pr][ch][:, mt * P:(mt + 1) * P]
                            nc.tensor.matmul(y_ps1[:], lhs, wo_sb[pr][:, 0:512],
                                             start=(pr == 0),
                                             stop=(pr == NPAIR - 1))
                        if last:
                            nc.scalar.copy(y_sb[:, 0:512], y_ps1[:])
                        else:
                            nc.vector.tensor_copy(y_sb[:, 0:512], y_ps1[:])
                        y_ps2 = p_aux.tile([P, C - 512], f32, tag="aux",
                                           name=f"yp2{ch}_{mt}")
                        for pr in range(NPAIR):
                            lhs = aT[pr][ch][:, mt * P:(mt + 1) * P]
                            nc.tensor.matmul(y_ps2[:], lhs, wo_sb[pr][:, 512:C],
                                             start=(pr == 0),
                                             stop=(pr == NPAIR - 1))
                        if last:
                            nc.scalar.copy(y_sb[:, 512:C], y_ps2[:])
                        else:
                            nc.vector.tensor_copy(y_sb[:, 512:C], y_ps2[:])
                        nc.sync.dma_start(y[row0:row0 + P, :], y_sb[:])

                for pr in range(NPAIR):
                    for ch in range(NCH):
                        emit_attn(pr, ch)
                        if ch == 0 and pr + 1 < NPAIR:
                            emit_proj(pr + 1)
                        if pr == NPAIR - 1:
                            emit_outproj(ch, last=(ch == NCH - 1))

    nc.compile()
    return nc


def _get_nc():
    if "nc" not in _CACHE:
        _CACHE["nc"] = _build()
    return _CACHE["nc"]


def make_in_maps(x, Wq, bq, Wk, Wv, bv, Wo):
    import ml_dtypes
    bf16 = ml_dtypes.bfloat16
    in_maps = []
    for c in range(8):
        b, g = c // 2, c % 2
        sel = slice(g * DV, (g + 1) * DV)
        in_maps.append({
            "xT": np.ascontiguousarray(x[b].T),
            "wqT": np.ascontiguousarray(Wq[sel, :].T),
            "wkT": np.ascontiguousarray(Wk[sel, :].T),
            "wvT": np.ascontiguousarray(Wv[sel, :].T),
            "woT": np.ascontiguousarray(Wo[:, sel].T),
            "bq": np.ascontiguousarray(bq[sel].reshape(DV, 1)),
            "bv": np.ascontiguousarray(bv[sel].reshape(1, DV)).astype(bf16),
        })
    return in_maps


def kernel(x, Wq, bq, Wk, bk, Wv, bv, Wo, bo, **_unused):
    from concourse.bass_utils import run_bass_kernel_spmd

    x = np.ascontiguousarray(np.asarray(x, dtype=np.float32))
    Wq = np.asarray(Wq, dtype=np.float32)
    Wk = np.asarray(Wk, dtype=np.float32)
    Wv = np.asarray(Wv, dtype=np.float32)
    Wo = np.asarray(Wo, dtype=np.float32)
    bq = np.asarray(bq, dtype=np.float32)
    bv = np.asarray(bv, dtype=np.float32)
    bo = np.asarray(bo, dtype=np.float32)

    in_maps = make_in_maps(x, Wq, bq, Wk, Wv, bv, Wo)
    nc = _get_nc()
    res = run_bass_kernel_spmd(nc, in_maps, core_ids=list(range(8)),
                               trace=bool(_CACHE.get("trace", False)))
    _CACHE["last_result"] = res

    out = np.empty((B, N, C), dtype=np.float32)
    for b in range(B):
        out[b] = res.results[2 * b]["y"] + res.results[2 * b + 1]["y"] + bo
    return out
